# revision 1
# baseline (speedup 1.0000x reference)
"""Trainium2 Bass kernel for the masked-attention block (nn_MAB_61607010894006).

Sharding: data-parallel over batch B=8 across 8 NeuronCores (one batch row
per core, weights replicated, no collectives).

Per-core strategy: activations live transposed ("feature-major",
[features, tokens]) so every matmul takes its natural operands:
  qT/kT      = W.T @ X.T      (lhsT = W chunk, rhs = XT chunk)
  S^T        = kT_h' @ qT_h   (k tokens on partitions, q tokens free)
  softmax    : exp on ScalarE with mask as per-partition bias (-1e9), no
               max-subtraction (scores are O(1)); normalization deferred:
  o^T        = [v | 1]' @ A^T accumulated over k tiles -> row 64 is the
               softmax denominator; multiply by its reciprocal afterwards.
  layernorm  : feature-dim (partition) sums via ones-column matmuls on PE;
               per-token stats broadcast back via tiny PE ones-matmuls.
  FC         = Wo' @ OT, relu+bias fused into the ScalarE eviction.

Mask compaction: only unmasked key tokens are shipped per core (masked ones
contribute exactly +0.0 to the softmax numerator and denominator — in the
fp32 reference too), padded to a 128 multiple.

All data stays fp32 in SBUF; matmul operands are bitcast to float32r, which
streams at bf16 rate for free dims >= 256 with ~tf32 precision.
"""

import sys

sys.path.insert(0, "/opt/trn_rl_repo")

import numpy as np

import concourse.bass as bass
import concourse.mybir as mybir
import concourse.tile as tile
from concourse.bass_utils import run_bass_kernel_spmd

F32 = mybir.dt.float32
F32R = mybir.dt.float32r
AF = mybir.ActivationFunctionType

B, NQ, NK, D, H, DH = 8, 1024, 1024, 512, 8, 64
EPS = 1e-5
NEG = -1e9
N_CORES = 8

# matmul operand dtype: float32r (full speed, ~tf32 precision) or float32
# (exact, 4 cycles/row).
MM = F32R


def _split_multi_waits(nc):
    """This toolchain's walrus allows ONE sem wait per TPB instruction; Tile
    can emit several (kernel-tail drain). Hoist extras onto preceding
    single-wait NOPs on the same engine stream (equivalent: in-order issue).
    """
    multi_update = []
    for fn in nc.m.functions:
        for bb in fn.blocks:
            insts = bb.instructions
            new = []
            changed = False
            for inst in insts:
                si = inst.sync_info
                if si is not None and si.on_wait and len(si.on_wait) > 1:
                    waits = list(si.on_wait)
                    for w in waits[:-1]:
                        nop = mybir.InstNoOp(
                            name=f"I-wsplit-{nc.next_id()}", engine=inst.engine
                        )
                        nop.sync_info = mybir.SyncInfo(on_wait=[w], on_update=[])
                        new.append(nop)
                    inst.sync_info = mybir.SyncInfo(
                        on_wait=[waits[-1]], on_update=list(si.on_update)
                    )
                    changed = True
                if si is not None and si.on_update and len(si.on_update) > 1:
                    multi_update.append(inst.name)
                new.append(inst)
            if changed:
                bb.instructions = new
    if multi_update:
        raise RuntimeError(f">1 sem update unsupported: {multi_update[:10]}")


def _act_recip(nc, out, in_):
    """ACT-table reciprocal via raw InstActivation (measured max rel err
    ~1e-5 on HW, inside this kernel's error budget)."""
    eng = nc.scalar
    inputs = [eng.lower_ap(in_)]
    for arg in (0.0, 1.0, 0.0):  # bias, scale, alpha
        inputs.append(mybir.ImmediateValue(dtype=mybir.dt.float32, value=arg))
    return eng.add_instruction(
        mybir.InstActivation(
            name=f"I-actrecip-{nc.next_id()}",
            func=AF.Reciprocal,
            ins=inputs,
            outs=[eng.lower_ap(out)],
        )
    )


def chunks(n, w=512):
    out, s = [], 0
    while s < n:
        out.append((s, min(w, n - s)))
        s += min(w, n - s)
    return out


def build_nc(mm=MM, kt_tiles=8):
    NKP = kt_tiles * 128  # compacted+padded key/value token count
    nc = bass.Bass()

    qt_d = nc.dram_tensor("qt", [D, NQ], mm, kind="ExternalInput")
    kt_d = nc.dram_tensor("kt", [D + 1, NKP], mm, kind="ExternalInput")  # +ones
    wq_d = nc.dram_tensor("wq", [D, D], mm, kind="ExternalInput")
    wk_d = nc.dram_tensor("wk", [D, D], mm, kind="ExternalInput")
    wv_d = nc.dram_tensor("wv", [D + 1, D], mm, kind="ExternalInput")  # +bv row
    wo_d = nc.dram_tensor("wo", [D, D], mm, kind="ExternalInput")
    bq_d = nc.dram_tensor("bq", [128, 4], F32, kind="ExternalInput")
    bk_d = nc.dram_tensor("bk", [128, 4], F32, kind="ExternalInput")
    bo_d = nc.dram_tensor("bo", [128, 4], F32, kind="ExternalInput")
    mb_d = nc.dram_tensor("mb", [128, kt_tiles], F32, kind="ExternalInput")
    gb_d = nc.dram_tensor("gb", [128, 16], F32, kind="ExternalInput")  # g0b0g1b1
    on_d = nc.dram_tensor("on", [128, 128], mm, kind="ExternalInput")  # all ones
    out_d = nc.dram_tensor("out", [D, NQ], F32, kind="ExternalOutput")

    mult, add = mybir.AluOpType.mult, mybir.AluOpType.add

    with tile.TileContext(nc) as tc:
        with (
            tc.tile_pool(name="wp", bufs=1) as wp,
            tc.tile_pool(name="ap", bufs=1) as ap,
            tc.tile_pool(name="sm", bufs=2) as sm,
            tc.tile_pool(name="pp", bufs=2, space="PSUM") as pp,
        ):
            # ---- weights ------------------------------------------------------
            wq_sb = wp.tile([128, 4 * D], mm, name="wq_sb")
            wk_sb = wp.tile([128, 4 * D], mm, name="wk_sb")
            wv_sb = wp.tile([128, 4 * D], mm, name="wv_sb")
            wv1_sb = wp.tile([1, D], mm, name="wv1_sb")
            wo_sb = wp.tile([128, 4 * D], mm, name="wo_sb")
            for t in range(4):
                nc.sync.dma_start(
                    wq_sb[:, t * D : (t + 1) * D], wq_d[t * 128 : (t + 1) * 128, :]
                )
                nc.sync.dma_start(
                    wk_sb[:, t * D : (t + 1) * D], wk_d[t * 128 : (t + 1) * 128, :]
                )
                nc.sync.dma_start(
                    wv_sb[:, t * D : (t + 1) * D], wv_d[t * 128 : (t + 1) * 128, :]
                )
                nc.sync.dma_start(
                    wo_sb[:, t * D : (t + 1) * D], wo_d[t * 128 : (t + 1) * 128, :]
                )
            nc.sync.dma_start(wv1_sb[:, :], wv_d[D : D + 1, :])

            bq_sb = wp.tile([128, 4], F32, name="bq_sb")
            bk_sb = wp.tile([128, 4], F32, name="bk_sb")
            bo_sb = wp.tile([128, 4], F32, name="bo_sb")
            mb_sb = wp.tile([128, kt_tiles], F32, name="mb_sb")
            gb_sb = wp.tile([128, 16], F32, name="gb_sb")
            nc.sync.dma_start(bq_sb[:], bq_d[:])
            nc.sync.dma_start(bk_sb[:], bk_d[:])
            nc.sync.dma_start(bo_sb[:], bo_d[:])
            nc.sync.dma_start(mb_sb[:], mb_d[:])
            nc.sync.dma_start(gb_sb[:], gb_d[:])

            ones_sb = wp.tile([128, 128], mm, name="ones_sb")
            nc.sync.dma_start(ones_sb[:], on_d[:])
            ones128 = ones_sb[:, 0:1]
            ones_r64 = ones_sb  # any row is ones; row 64 used for lane-64 ops
            ones_r0 = ones_sb
            eps_sb = wp.tile([1, 1], F32, name="eps_sb")
            nc.vector.memset(eps_sb[:], EPS)

            kt1_sb = wp.tile([1, NKP], mm, name="kt1_sb")
            nc.sync.dma_start(kt1_sb[:, :], kt_d[D : D + 1, :])

            # ---- persistent activations --------------------------------------
            q_f32 = ap.tile([128, 4 * NQ], mm, name="q_f32")
            k_f32 = ap.tile([128, 4 * NKP], mm, name="k_f32", tag="kmm_sq")
            v_sb = ap.tile([128, kt_tiles * (8 * 65)], mm, name="v_sb")
            v_ones = v_sb.rearrange("p (v h x) -> p v h x", v=kt_tiles, h=8)[
                :, :, :, 64
            ]
            nc.vector.tensor_copy(
                v_ones, ones_sb[:, 0:1].rearrange("p (a b) -> p a b", a=1)
                .broadcast_to([128, kt_tiles, 8])
            )

            # ---- staging (released after phase 1) ----------------------------
            with tc.tile_pool(name="stg", bufs=1) as stg:
                qt_sb = stg.tile([128, 4 * NQ], mm, name="qt_sb")
                kt_sb = stg.tile([128, 4 * NKP], mm, name="kt_sb")
                for t in range(4):
                    nc.sync.dma_start(
                        qt_sb[:, t * NQ : (t + 1) * NQ],
                        qt_d[t * 128 : (t + 1) * 128, :],
                    )
                    nc.sync.dma_start(
                        kt_sb[:, t * NKP : (t + 1) * NKP],
                        kt_d[t * 128 : (t + 1) * 128, :],
                    )

                # ---- phase 1: projections ------------------------------------
                for t in range(4):
                    for cs, cw in chunks(NQ):
                        ps_q = pp.tile([128, 512], F32, name="ps_q", tag="pp")
                        for kc in range(4):
                            nc.tensor.matmul(
                                ps_q[:, 0:cw],
                                (wq_sb[:, kc * D + t * 128 : kc * D + (t + 1) * 128]),
                                (qt_sb[:, kc * NQ + cs : kc * NQ + cs + cw]),
                                start=(kc == 0),
                                stop=(kc == 3),
                            )
                        dst = slice(t * NQ + cs, t * NQ + cs + cw)
                        nc.scalar.activation(
                            q_f32[:, dst], ps_q[:, 0:cw], AF.Identity,
                            bias=bq_sb[:, t : t + 1],
                        )
                    for cs, cw in chunks(NKP):
                        ps_k = pp.tile([128, 512], F32, name="ps_k", tag="pp")
                        for kc in range(4):
                            nc.tensor.matmul(
                                ps_k[:, 0:cw],
                                (wk_sb[:, kc * D + t * 128 : kc * D + (t + 1) * 128]),
                                (kt_sb[:, kc * NKP + cs : kc * NKP + cs + cw]),
                                start=(kc == 0),
                                stop=(kc == 3),
                            )
                        dst = slice(t * NKP + cs, t * NKP + cs + cw)
                        nc.scalar.activation(
                            k_f32[:, dst], ps_k[:, 0:cw], AF.Identity,
                            bias=bk_sb[:, t : t + 1],
                        )

                # v token-major [NKP, 512] (+bias via augmented ones row)
                for vt in range(kt_tiles):
                    ps_v = pp.tile([128, 512], F32, name="ps_v", tag="pp")
                    for kc in range(4):
                        nc.tensor.matmul(
                            ps_v[:],
                            (kt_sb[:, kc * NKP + vt * 128 : kc * NKP + (vt + 1) * 128]),
                            (wv_sb[:, kc * D : (kc + 1) * D]),
                            start=(kc == 0),
                            stop=False,
                        )
                    nc.tensor.matmul(
                        ps_v[:],
                        (kt1_sb[0:1, vt * 128 : (vt + 1) * 128]),
                        (wv1_sb[0:1, :]),
                        start=False,
                        stop=True,
                    )
                    v_dst = v_sb[:, vt * 520 : (vt + 1) * 520].rearrange(
                        "p (h x) -> p h x", h=8
                    )[:, :, 0:64]
                    v_src = ps_v.rearrange("p (h x) -> p h x", h=8)
                    nc.scalar.copy(v_dst, v_src)

            # ---- phase 2: attention ------------------------------------------
            o_f32 = ap.tile([128, 4 * NQ], mm, name="o_f32", tag="bigf32", bufs=2)
            for h in range(H):
                pr, rh = h // 2, (h % 2) * 64
                at_tiles = []
                for i in range(kt_tiles):
                    ps_s = pp.tile([128, NQ], F32, name="ps_s", tag="ps")
                    for c in range(2):
                        nc.tensor.matmul(
                            ps_s[:, c * 512 : (c + 1) * 512],
                            (k_f32[rh : rh + 64,
                                    pr * NKP + i * 128 : pr * NKP + (i + 1) * 128]),
                            (q_f32[rh : rh + 64,
                                    pr * NQ + c * 512 : pr * NQ + (c + 1) * 512]),
                            start=True,
                            stop=True,
                        )
                    at_sb = ap.tile([128, NQ], mm, name="at_sb", tag="at", bufs=8)
                    at_tiles.append(at_sb)
                    nc.scalar.activation(
                        at_sb[:, :], ps_s[:, :], AF.Exp,
                        bias=mb_sb[:, i : i + 1], scale=0.125,
                    )
                for c in range(2):
                    po = pp.tile([65, 512], F32, name="po", tag="po")
                    for i in range(kt_tiles):
                        nc.tensor.matmul(
                            po[:],
                            (v_sb[:, i * 520 + h * 65 : i * 520 + (h + 1) * 65]),
                            (at_tiles[i][:, c * 512 : (c + 1) * 512]),
                            start=(i == 0),
                            stop=(i == kt_tiles - 1),
                        )
                    # softmax denominator: po row 64 (lane 64); reciprocal on
                    # lane 64, PE-broadcast to lanes 0..63, normalize there.
                    rinv = sm.tile([65, 512], mm, name="rinv", tag="rinv")
                    _act_recip(nc, rinv[64:65, :], po[64:65, :])
                    pb = pp.tile([64, 512], F32, name="pb", tag="pp")
                    nc.tensor.matmul(
                        pb[:], (ones_r64[64:65, 0:64]), (rinv[64:65, :]),
                        start=True, stop=True,
                    )
                    rb = sm.tile([64, 512], F32, name="rb", tag="rb")
                    nc.vector.tensor_copy(rb[:, :], pb[:, :])
                    avn = sm.tile([64, 512], F32, name="avn", tag="avn")
                    nc.vector.tensor_mul(avn[:, :], po[0:64, :], rb[:, :])
                    qsl = slice(pr * NQ + c * 512, pr * NQ + (c + 1) * 512)
                    if rh == 0:
                        nc.vector.tensor_add(
                            o_f32[0:64, qsl], avn[:, :], q_f32[0:64, qsl]
                        )
                    else:
                        # odd head: shift Av/r to lanes 64..127 (PSUM is not
                        # DMA-readable; shift the normalized SBUF copy)
                        av2 = sm.tile([128, 512], F32, name="av2", tag="rb")
                        nc.gpsimd.dma_start(av2[64:128, :], avn[:, :])
                        nc.vector.tensor_add(
                            o_f32[64:128, qsl], av2[64:128, :], q_f32[64:128, qsl]
                        )

            # ---- layernorm helper --------------------------------------------
            def layer_norm(x_f32, gcol, bcol, out_f32):
                sq = ap.tile([128, 4 * NQ], mm, name="sq", tag="kmm_sq")
                for t in range(4):
                    sl = slice(t * NQ, (t + 1) * NQ)
                    nc.vector.tensor_mul(sq[:, sl], x_f32[:, sl], x_f32[:, sl])
                mu = sm.tile([1, NQ], mm, name="mu", tag="mu", bufs=1)
                ex2 = sm.tile([1, NQ], F32, name="ex2", tag="ex2", bufs=1)
                for c in range(2):
                    ps_su = pp.tile([1, 512], F32, name="ps_su", tag="po")
                    ps_sq = pp.tile([1, 512], F32, name="ps_sq", tag="po")
                    for t in range(4):
                        sl = slice(t * NQ + c * 512, t * NQ + (c + 1) * 512)
                        nc.tensor.matmul(
                            ps_su[:], (ones128), (x_f32[:, sl]),
                            start=(t == 0), stop=(t == 3),
                        )
                        nc.tensor.matmul(
                            ps_sq[:], (ones128), (sq[:, sl]),
                            start=(t == 0), stop=(t == 3),
                        )
                    csl = slice(c * 512, (c + 1) * 512)
                    nc.vector.tensor_scalar_mul(mu[:, csl], ps_su[:], 1.0 / D)
                    nc.vector.tensor_scalar_mul(ex2[:, csl], ps_sq[:], 1.0 / D)
                var = sm.tile([1, NQ], F32, name="var", tag="var", bufs=1)
                nc.vector.tensor_mul(var[:], mu[:], mu[:])
                nc.vector.tensor_sub(var[:], ex2[:], var[:])
                nc.scalar.activation(var[:], var[:], AF.Sqrt, bias=eps_sb[0:1, 0:1])
                rstd = sm.tile([1, NQ], mm, name="rstd", tag="rstd", bufs=1)
                _act_recip(nc, rstd[:], var[:])
                # mur = mu * rstd, in place over mu
                nc.vector.tensor_mul(mu[:], mu[:], rstd[:])
                rstd_rep = ap.tile([128, NQ], F32, name="rstd_rep", tag="rep", bufs=2)
                mur_rep = ap.tile([128, NQ], F32, name="mur_rep", tag="rep", bufs=2)
                for c in range(2):
                    csl = slice(c * 512, (c + 1) * 512)
                    pb1 = pp.tile([128, 512], F32, name="pb1", tag="pp")
                    nc.tensor.matmul(
                        pb1[:], (ones_r0[0:1, :]), (rstd[0:1, csl]),
                        start=True, stop=True,
                    )
                    nc.vector.tensor_copy(rstd_rep[:, csl], pb1[:])
                    pb2 = pp.tile([128, 512], F32, name="pb2", tag="pp")
                    nc.tensor.matmul(
                        pb2[:], (ones_r0[0:1, :]), (mu[0:1, csl]),
                        start=True, stop=True,
                    )
                    nc.vector.tensor_copy(mur_rep[:, csl], pb2[:])
                for t in range(4):
                    sl = slice(t * NQ, (t + 1) * NQ)
                    nc.vector.tensor_mul(out_f32[:, sl], x_f32[:, sl], rstd_rep[:])
                    nc.vector.tensor_sub(out_f32[:, sl], out_f32[:, sl], mur_rep[:])
                    nc.vector.tensor_scalar(
                        out_f32[:, sl], out_f32[:, sl],
                        gb_sb[:, gcol + t : gcol + t + 1],
                        gb_sb[:, bcol + t : bcol + t + 1],
                        mult, add,
                    )

            # ---- phase 3: LN0 -------------------------------------------------
            ot0_f32 = ap.tile([128, 4 * NQ], mm, name="ot0_f32", tag="bigf32",
                              bufs=2)
            layer_norm(o_f32, 0, 4, ot0_f32)

            # ---- phase 4: FC + relu + residual -------------------------------
            o1_f32 = ap.tile([128, 4 * NQ], mm, name="o1_f32", tag="bigf32",
                             bufs=2)
            for ot in range(4):
                for c in range(2):
                    ps_f = pp.tile([128, 512], F32, name="ps_f", tag="pp")
                    for ft in range(4):
                        nc.tensor.matmul(
                            ps_f[:],
                            (wo_sb[:, ft * D + ot * 128 : ft * D + (ot + 1) * 128]),
                            (ot0_f32[:, ft * NQ + c * 512 : ft * NQ + (c + 1) * 512]),
                            start=(ft == 0),
                            stop=(ft == 3),
                        )
                    rl = sm.tile([128, 512], F32, name="rl", tag="avn")
                    nc.vector.tensor_scalar(
                        rl[:], ps_f[:], bo_sb[:, ot : ot + 1], 0.0,
                        mybir.AluOpType.add, mybir.AluOpType.max,
                    )
                    sl = slice(ot * NQ + c * 512, ot * NQ + (c + 1) * 512)
                    nc.vector.tensor_add(o1_f32[:, sl], ot0_f32[:, sl], rl[:])

            # ---- phase 5: LN1 -> out ------------------------------------------
            otout = ap.tile([128, 4 * NQ], F32, name="otout", tag="bigf32", bufs=2)
            layer_norm(o1_f32, 8, 12, otout)

            for t in range(4):
                nc.sync.dma_start(
                    out_d[t * 128 : (t + 1) * 128, :], otout[:, t * NQ : (t + 1) * NQ]
                )

    _split_multi_waits(nc)
    return nc


_nc_cache = {}


def _get_nc(mm=MM, kt_tiles=8):
    key = (str(mm), kt_tiles)
    if key not in _nc_cache:
        _nc_cache[key] = build_nc(mm, kt_tiles)
    return _nc_cache[key]


def _kt_tiles_for(mask):
    n = int(max(int((mask[b] != 0).sum()) for b in range(mask.shape[0])))
    return max(1, (n + 127) // 128)


def prep_inputs(Q, K, mask, Wq, bq, Wk, bk, Wv, bv, Wo, bo, g0, b0, g1, b1, mm=MM,
                kt_tiles=None):
    f32 = np.float32
    ones_h = np.ones((128, 128), f32)
    if kt_tiles is None:
        kt_tiles = _kt_tiles_for(mask)
    nkp = kt_tiles * 128

    def percol(v):  # [512] feature vector -> [128, 4] per-partition layout
        return np.ascontiguousarray(np.asarray(v, f32).reshape(4, 128).T)

    wv_h = np.ascontiguousarray(
        np.vstack([np.asarray(Wv, f32), np.asarray(bv, f32)[None, :]])
    )
    gb = np.concatenate([percol(g0), percol(b0), percol(g1), percol(b1)], axis=1)
    wq_h = np.ascontiguousarray(np.asarray(Wq, f32))
    wk_h = np.ascontiguousarray(np.asarray(Wk, f32))
    wo_h = np.ascontiguousarray(np.asarray(Wo, f32))

    in_maps = []
    for b in range(B):
        qt = np.ascontiguousarray(np.asarray(Q[b], f32).T)
        idx = np.nonzero(mask[b] != 0)[0]
        kc = np.zeros((nkp, D), f32)
        kc[: len(idx)] = np.asarray(K[b], f32)[idx]
        kt = np.ascontiguousarray(np.vstack([kc.T, np.ones((1, nkp), f32)]))
        mb = np.full(nkp, np.float32(NEG))
        mb[: len(idx)] = 0.0
        mb = np.ascontiguousarray(mb.reshape(kt_tiles, 128).T.astype(f32))
        in_maps.append(
            {
                "qt": qt,
                "kt": kt,
                "wq": wq_h,
                "wk": wk_h,
                "wv": wv_h,
                "wo": wo_h,
                "bq": percol(bq),
                "bk": percol(bk),
                "bo": percol(bo),
                "mb": mb,
                "gb": gb,
                "on": ones_h,
            }
        )
    return in_maps


def kernel(Q, K, mask, Wq, bq, Wk, bk, Wv, bv, Wo, bo, g0, b0, g1, b1):
    mask = np.asarray(mask)
    kt_tiles = _kt_tiles_for(mask)
    nc = _get_nc(MM, kt_tiles)
    in_maps = prep_inputs(
        Q, K, mask, Wq, bq, Wk, bk, Wv, bv, Wo, bo, g0, b0, g1, b1, MM, kt_tiles
    )
    res = run_bass_kernel_spmd(nc, in_maps, list(range(N_CORES)))
    out = np.stack(
        [np.ascontiguousarray(res.results[i]["out"].T) for i in range(N_CORES)]
    )
    return out.astype(np.float32)



# revision 10
# speedup vs baseline: 1.2933x; 1.2933x over previous
"""Trainium2 Bass kernel for the masked-attention block (nn_MAB_61607010894006).

Sharding: data-parallel over batch B=8 across 8 NeuronCores (one batch row
per core, weights replicated, no collectives).

v2 design (vs the 249us v1 baseline):
  - bf16 activations+weights (fp32 PSUM accumulation, fp32 LN stats, fp32
    output). Halves DMA, enables FWL on LDWEIGHTS and DVE 2x perf modes.
  - Scores: one matmul per (head, k-tile) with free dim 1024; head PAIRS
    run concurrently on the PE via row tiling (contraction=64: heads
    2t/2t+1 at array rows 0-63/64-127).
  - Masking without exp-bias: masked k tokens compacted out on the host;
    padded k columns are zero (scores=0, exp=1) and both the v rows and
    the denominator indicator row are 0 there, so pads contribute nothing
    to numerator or denominator.
  - Softmax denominator rides the po matmul as a 65th v row (indicator).
    ALL reciprocals/rsqrts are computed as exp(-ln(x)) / exp(-0.5 ln(x))
    on ScalarE: Ln and Exp share one ACT table set, so the kernel does a
    single ACT_TABLE_LOAD (v1 lost 26us to exp<->recip table thrash).
    (Custom DVE ops and gpsimd compute ops fail codegen in this
    toolchain - only gpsimd DMA works.)
  - Row->partitions broadcasts via tiny PE ones-matmuls; elementwise on
    DVE in bf16 (2x mode); odd-head partition shifts via gpsimd DMA.
"""

import sys

sys.path.insert(0, "/opt/trn_rl_repo")

import numpy as np
import ml_dtypes

import concourse.bass as bass
import concourse.mybir as mybir
import concourse.tile as tile
from concourse.bass_utils import run_bass_kernel_spmd

F32 = mybir.dt.float32
F32R = mybir.dt.float32r
BF16 = mybir.dt.bfloat16
AF = mybir.ActivationFunctionType

B, NQ, NK, D, H, DH = 8, 1024, 1024, 512, 8, 64
EPS = 1e-5
N_CORES = 8


def _split_multi_waits(nc):
    """This toolchain's walrus allows ONE sem wait per TPB instruction; Tile
    can emit several (kernel-tail drain). Hoist extras onto preceding
    single-wait NOPs on the same engine stream (equivalent: in-order issue).
    """
    multi_update = []
    for fn in nc.m.functions:
        for bb in fn.blocks:
            insts = bb.instructions
            new = []
            changed = False
            for inst in insts:
                si = inst.sync_info
                if si is not None and si.on_wait and len(si.on_wait) > 1:
                    waits = list(si.on_wait)
                    for w in waits[:-1]:
                        nop = mybir.InstNoOp(
                            name=f"I-wsplit-{nc.next_id()}", engine=inst.engine
                        )
                        nop.sync_info = mybir.SyncInfo(on_wait=[w], on_update=[])
                        new.append(nop)
                    inst.sync_info = mybir.SyncInfo(
                        on_wait=[waits[-1]], on_update=list(si.on_update)
                    )
                    changed = True
                if si is not None and si.on_update and len(si.on_update) > 1:
                    multi_update.append(inst.name)
                new.append(inst)
            if changed:
                bb.instructions = new
    if multi_update:
        raise RuntimeError(f">1 sem update unsupported: {multi_update[:10]}")


def build_nc(kt_tiles=5, affine=False):
    NKP = kt_tiles * 128  # compacted+padded key/value token count
    nc = bass.Bass()

    qt_d = nc.dram_tensor("qt", [D, NQ], BF16, kind="ExternalInput")
    kt_d = nc.dram_tensor("kt", [D + 1, NKP], BF16, kind="ExternalInput")  # +ind
    wq_d = nc.dram_tensor("wq", [D, D], BF16, kind="ExternalInput")
    wk_d = nc.dram_tensor("wk", [D, D], BF16, kind="ExternalInput")
    wv_d = nc.dram_tensor("wv", [D + 1, D], BF16, kind="ExternalInput")  # +bv row
    wo_d = nc.dram_tensor("wo", [D, D], BF16, kind="ExternalInput")
    bq_d = nc.dram_tensor("bq", [128, 4], F32, kind="ExternalInput")
    bk_d = nc.dram_tensor("bk", [128, 4], F32, kind="ExternalInput")
    bo_d = nc.dram_tensor("bo", [128, 4], F32, kind="ExternalInput")
    ind_d = nc.dram_tensor("ind", [128, kt_tiles], BF16, kind="ExternalInput")
    cr_d = nc.dram_tensor("cr", [2, 128], BF16, kind="ExternalInput")  # row0=ones
    gb_d = nc.dram_tensor("gb", [128, 16], F32, kind="ExternalInput")  # percol
    cn_d = nc.dram_tensor("cn", [128, 1], BF16, kind="ExternalInput")  # 1/512
    out_d = nc.dram_tensor("out", [D, NQ], F32, kind="ExternalOutput")

    mult, add = mybir.AluOpType.mult, mybir.AluOpType.add

    def mm(out, lhsT, rhs, **kw):
        nc.tensor.matmul(out, lhsT, rhs, **kw)

    with tile.TileContext(nc) as tc:
        with (
            tc.tile_pool(name="wp", bufs=1) as wp,
            tc.tile_pool(name="ap", bufs=1) as ap,
            tc.tile_pool(name="sm", bufs=2) as sm,
            tc.tile_pool(name="pp", bufs=1, space="PSUM") as pp,
        ):
            # ---- small constants first (cheap DMAs) ---------------------------
            bq_sb = wp.tile([128, 4], F32, name="bq_sb")
            bk_sb = wp.tile([128, 4], F32, name="bk_sb")
            bo_sb = wp.tile([128, 4], F32, name="bo_sb")
            ind_sb = wp.tile([128, kt_tiles], BF16, name="ind_sb")
            cr_sb = wp.tile([2, 128], BF16, name="cr_sb")
            gb_sb = wp.tile([128, 16], F32, name="gb_sb")
            cn_sb = wp.tile([128, 1], BF16, name="cn_sb")
            nc.sync.dma_start(bq_sb[:], bq_d[:])
            nc.sync.dma_start(bk_sb[:], bk_d[:])
            nc.sync.dma_start(bo_sb[:], bo_d[:])
            nc.sync.dma_start(ind_sb[:], ind_d[:])
            nc.sync.dma_start(cr_sb[:], cr_d[:])
            nc.sync.dma_start(gb_sb[:], gb_d[:])
            nc.sync.dma_start(cn_sb[:], cn_d[:])
            eps_sb = wp.tile([1, 1], F32, name="eps_sb")
            nc.vector.memset(eps_sb[:], EPS)
            sum_lhs = cn_sb[:, 0:1]                    # [128,1] bf16 = 1/512
            ones_row = cr_sb[0:1, :]                   # [1,128] bf16 lhsT

            # ---- PE warmup while DMAs stream ---------------------------------
            wu_sb = wp.tile([128, 128], BF16, name="wu_sb")
            nc.vector.memset(wu_sb[:], 0.001)
            wu_ps = pp.tile([128, 512], F32, name="wu_ps", tag="s1", bufs=2)
            for i in range(12):
                mm(wu_ps[:, 0:128], wu_sb[:], wu_sb[:],
                   start=(i == 0), stop=(i == 11))
            wu_out = wp.tile([1, 1], F32, name="wu_out")
            nc.vector.tensor_copy(wu_out[:], wu_ps[0:1, 0:1])

            # ---- weights ------------------------------------------------------
            wq_sb = wp.tile([128, 4 * D], BF16, name="wq_sb")
            wk_sb = wp.tile([128, 4 * D], BF16, name="wk_sb")
            wv_sb = wp.tile([128, 4 * D], BF16, name="wv_sb")
            wv1_sb = wp.tile([1, D], BF16, name="wv1_sb")
            wo_sb = wp.tile([128, 4 * D], BF16, name="wo_sb")
            for t in range(4):
                nc.sync.dma_start(
                    wv_sb[:, t * D : (t + 1) * D], wv_d[t * 128 : (t + 1) * 128, :]
                )
            nc.sync.dma_start(wv1_sb[:, :], wv_d[D : D + 1, :])

            # ---- staged inputs (kt first: v+k projections start earliest) ----
            kt_sb = wp.tile([128, 4 * NKP], BF16, name="kt_sb")
            kt1_sb = wp.tile([1, NKP], BF16, name="kt1_sb")
            for t in range(4):
                nc.sync.dma_start(
                    kt_sb[:, t * NKP : (t + 1) * NKP],
                    kt_d[t * 128 : (t + 1) * 128, :],
                )
            nc.sync.dma_start(kt1_sb[:, :], kt_d[D : D + 1, :])
            for t in range(4):
                nc.sync.dma_start(
                    wk_sb[:, t * D : (t + 1) * D], wk_d[t * 128 : (t + 1) * 128, :]
                )
            qt_sb = wp.tile([128, 4 * NQ], BF16, name="qt_sb")
            for t in range(4):
                nc.sync.dma_start(
                    qt_sb[:, t * NQ : (t + 1) * NQ],
                    qt_d[t * 128 : (t + 1) * 128, :],
                )
            for t in range(4):
                nc.sync.dma_start(
                    wq_sb[:, t * D : (t + 1) * D], wq_d[t * 128 : (t + 1) * 128, :]
                )
            for t in range(4):
                nc.sync.dma_start(
                    wo_sb[:, t * D : (t + 1) * D], wo_d[t * 128 : (t + 1) * 128, :]
                )

            # ---- persistent activations --------------------------------------
            q_bf = ap.tile([128, 4 * NQ], BF16, name="q_bf")
            k_bf = ap.tile([128, 4 * NKP], BF16, name="k_bf")
            # v: per k-tile vt: 8 head blocks of 65 (64 values + indicator col)
            v_sb = ap.tile([128, kt_tiles * (8 * 65)], BF16, name="v_sb")
            v_ones = v_sb.rearrange("p (v h x) -> p v h x", v=kt_tiles, h=8)[
                :, :, :, 64
            ]
            nc.vector.tensor_copy(
                v_ones,
                ind_sb.rearrange("p (v a) -> p v a", a=1)
                .broadcast_to([128, kt_tiles, 8]),
            )

            # ---- phase 1a: v projection (token-major, +bias via ind row) -----
            for vt in range(kt_tiles):
                ps_v = pp.tile([128, 512], F32, name="ps_v", tag="s1", bufs=2)
                for kc in range(4):
                    mm(
                        ps_v[:],
                        kt_sb[:, kc * NKP + vt * 128 : kc * NKP + (vt + 1) * 128],
                        wv_sb[:, kc * D : (kc + 1) * D],
                        start=(kc == 0),
                        stop=False,
                    )
                mm(
                    ps_v[:],
                    kt1_sb[0:1, vt * 128 : (vt + 1) * 128],
                    wv1_sb[0:1, :],
                    start=False,
                    stop=True,
                )
                v_dst = v_sb[:, vt * 520 : (vt + 1) * 520].rearrange(
                    "p (h x) -> p h x", h=8
                )[:, :, 0:64]
                nc.scalar.copy(v_dst, ps_v.rearrange("p (h x) -> p h x", h=8))

            # ---- phases 1b+2 interleaved per t-block -------------------------
            o_bf = ap.tile([128, 4 * NQ], BF16, name="o_bf")
            sqt_tiles = []
            kchunks = [(0, 512), (512, NKP - 512)] if NKP > 512 else [(0, NKP)]

            for t in range(4):
                tsl = slice(t * NQ, (t + 1) * NQ)
                # -- k projection block t (kc-outer, weights reused) --
                ps_k = [
                    pp.tile([128, cw], F32, name=f"ps_k{t}_{ci}", tag="s1", bufs=2)
                    for ci, (cs, cw) in enumerate(kchunks)
                ]
                for kc in range(4):
                    for ci, (cs, cw) in enumerate(kchunks):
                        mm(
                            ps_k[ci][:],
                            wk_sb[:, kc * D + t * 128 : kc * D + (t + 1) * 128],
                            kt_sb[:, kc * NKP + cs : kc * NKP + cs + cw],
                            start=(kc == 0),
                            stop=(kc == 3),
                        )
                for ci, (cs, cw) in enumerate(kchunks):
                    nc.vector.tensor_scalar_add(
                        k_bf[:, t * NKP + cs : t * NKP + cs + cw],
                        ps_k[ci][:],
                        bk_sb[:, t : t + 1],
                    )
                # -- q projection block t --
                ps_q = [
                    pp.tile([128, 512], F32, name=f"ps_q{t}_{c}", tag="s1", bufs=2)
                    for c in range(2)
                ]
                for kc in range(4):
                    for c in range(2):
                        mm(
                            ps_q[c][:],
                            wq_sb[:, kc * D + t * 128 : kc * D + (t + 1) * 128],
                            qt_sb[:, kc * NQ + c * 512 : kc * NQ + c * 512 + 512],
                            start=(kc == 0),
                            stop=(kc == 3),
                        )
                for c in range(2):
                    nc.vector.tensor_scalar_add(
                        q_bf[:, t * NQ + c * 512 : t * NQ + c * 512 + 512],
                        ps_q[c][:],
                        bq_sb[:, t : t + 1],
                    )

                # -- scores + exp for head pair (2t, 2t+1), row-tiled --
                at_tiles = {0: [], 1: []}
                for i in range(kt_tiles):
                    for sub in range(2):
                        rh = sub * 64
                        ps_s = pp.tile(
                            [128, NQ], F32, name=f"s{t}_{i}_{sub}", tag="sx",
                            bufs=2,
                        )
                        for c in range(2):
                            mm(
                                ps_s[:, c * 512 : (c + 1) * 512],
                                k_bf[rh : rh + 64,
                                     t * NKP + i * 128 : t * NKP + (i + 1) * 128],
                                q_bf[rh : rh + 64,
                                     t * NQ + c * 512 : t * NQ + c * 512 + 512],
                                start=True,
                                stop=True,
                            )
                        at_sb = ap.tile(
                            [128, NQ], BF16, name=f"at{t}_{i}_{sub}", tag="at",
                            bufs=12,
                        )
                        at_tiles[sub].append(at_sb)
                        nc.scalar.activation(at_sb[:], ps_s[:], AF.Exp,
                                             scale=0.125)

                # -- A@V + normalize + residual, per head, per q-chunk --
                for sub in range(2):
                    h = 2 * t + sub
                    rh = sub * 64
                    for c in range(2):
                        po = pp.tile([65, 512], F32, name=f"po{h}_{c}",
                                     tag="po1", bufs=2)
                        for i in range(kt_tiles):
                            mm(
                                po[:],
                                v_sb[:, i * 520 + h * 65 : i * 520 + (h + 1) * 65],
                                at_tiles[sub][i][:, c * 512 : (c + 1) * 512],
                                start=(i == 0),
                                stop=(i == kt_tiles - 1),
                            )
                        lnr = sm.tile([1, 512], F32, name=f"ln{h}{c}",
                                      tag="ri", bufs=2)
                        nc.scalar.activation(lnr[:], po[64:65, :], AF.Ln)
                        rinv = sm.tile([1, 512], BF16, name=f"r{h}{c}",
                                       tag="ri2", bufs=2)
                        nc.scalar.activation(rinv[:], lnr[:], AF.Exp,
                                             scale=-1.0)
                        pb = pp.tile([64, 512], F32, name=f"pb{h}{c}",
                                     tag="po1", bufs=2)
                        mm(pb[:], cr_sb[0:1, 0:64], rinv[:],
                           start=True, stop=True)
                        rb = sm.tile([64, 512], BF16, name=f"rb{h}{c}",
                                     tag="rb", bufs=2)
                        nc.vector.tensor_copy(rb[:], pb[:])
                        avn = sm.tile([64, 512], BF16, name=f"av{h}{c}",
                                      tag="av", bufs=2)
                        nc.vector.tensor_mul(avn[:], po[0:64, :], rb[:])
                        csl = slice(t * NQ + c * 512, t * NQ + c * 512 + 512)
                        if rh == 0:
                            nc.vector.tensor_add(
                                o_bf[0:64, csl], avn[:], q_bf[0:64, csl]
                            )
                        else:
                            av2 = sm.tile([128, 512], BF16, name=f"av2_{h}{c}",
                                          tag="av2", bufs=2)
                            nc.gpsimd.dma_start(av2[64:128, :], avn[:])
                            nc.vector.tensor_add(
                                o_bf[64:128, csl], av2[64:128, :],
                                q_bf[64:128, csl],
                            )

                # square of o block t for LN0 sumsq (sums deferred)
                sqt = sm.tile([128, NQ], BF16, name=f"sqt{t}", tag="sqt", bufs=4)
                sqt_tiles.append(sqt)
                nc.vector.tensor_mul(sqt[:], o_bf[:, tsl], o_bf[:, tsl])

            # ---- layernorm sums + stats + broadcast helper --------------------
            def ln_sums(x_bf, sq_tiles, tag):
                """Post-phase LN sums: per c-chunk one [65,512] f32 PSUM tile
                (mean row 0, mean-square row 64 via column tile position)."""
                sts = []
                for c in range(2):
                    st = pp.tile([65, 512], F32, name=f"st{tag}{c}", tag="sx",
                                 bufs=2)
                    for t in range(4):
                        csl = slice(t * NQ + c * 512, t * NQ + c * 512 + 512)
                        mm(st[0:1, :], sum_lhs, x_bf[:, csl],
                           start=(t == 0), stop=(t == 3))
                        mm(st[64:65, :], sum_lhs,
                           sq_tiles[t][:, c * 512 : c * 512 + 512],
                           start=(t == 0), stop=(t == 3))
                    sts.append(st)
                return sts

            def ln_stats_and_reps(sts, tag):
                """sts: per-c [65,512] f32 PSUM (mean row 0, meansq row 64).
                Returns (rr, rm): [128, NQ] bf16 broadcasts of rstd, mu*rstd."""
                mu = sm.tile([1, NQ], F32, name=f"mu{tag}", tag="mu", bufs=2)
                var = sm.tile([1, NQ], F32, name=f"var{tag}", tag="var", bufs=2)
                for c in range(2):
                    csl = slice(c * 512, (c + 1) * 512)
                    nc.vector.tensor_copy(mu[:, csl], sts[c][0:1, :])
                    nc.scalar.activation(var[:, csl], sts[c][0:1, :], AF.Square)
                    nc.vector.tensor_sub(var[:, csl], sts[c][64:65, :],
                                         var[:, csl])
                # rstd = exp(-0.5*ln(var+eps)) — stays in the Exp table set
                nc.scalar.activation(var[:], var[:], AF.Ln,
                                     bias=eps_sb[0:1, 0:1])
                rstd = sm.tile([1, NQ], BF16, name=f"rs{tag}", tag="rs", bufs=2)
                nc.scalar.activation(rstd[:], var[:], AF.Exp, scale=-0.5)
                murm = sm.tile([1, NQ], BF16, name=f"mm{tag}", tag="mm2", bufs=2)
                nc.vector.tensor_mul(murm[:], mu[:], rstd[:])
                rr = sm.tile([128, NQ], BF16, name=f"rrb{tag}", tag="rrb", bufs=2)
                rm = sm.tile([128, NQ], BF16, name=f"rmb{tag}", tag="rmb", bufs=2)
                for c in range(2):
                    csl = slice(c * 512, (c + 1) * 512)
                    rr_ps = pp.tile([128, 512], F32, name=f"rr{tag}{c}",
                                    tag="sx", bufs=2)
                    mm(rr_ps[:], ones_row, rstd[:, csl],
                       start=True, stop=True)
                    nc.vector.tensor_copy(rr[:, csl], rr_ps[:])
                    rm_ps = pp.tile([128, 512], F32, name=f"rm{tag}{c}",
                                    tag="sx", bufs=2)
                    mm(rm_ps[:], ones_row, murm[:, csl],
                       start=True, stop=True)
                    nc.vector.tensor_copy(rm[:, csl], rm_ps[:])
                return rr, rm

            # ---- phase 3: LN0 -------------------------------------------------
            ot0 = ap.tile([128, 4 * NQ], BF16, name="ot0")
            sts0 = ln_sums(o_bf, sqt_tiles, "l0")
            rr0, rm0 = ln_stats_and_reps(sts0, "l0")
            for t in range(4):
                sl = slice(t * NQ, (t + 1) * NQ)
                nc.vector.tensor_mul(ot0[:, sl], o_bf[:, sl], rr0[:])
                nc.vector.tensor_sub(ot0[:, sl], ot0[:, sl], rm0[:])
                if affine:
                    nc.vector.tensor_scalar(
                        ot0[:, sl], ot0[:, sl],
                        gb_sb[:, 0 + t : 0 + t + 1], gb_sb[:, 4 + t : 4 + t + 1],
                        mult, add,
                    )

            # ---- phase 4: FC + relu + residual; LN1 sums interleaved ---------
            o1 = ap.tile([128, 4 * NQ], BF16, name="o1")
            sq1_tiles = []
            for ot in range(4):
                osl = slice(ot * NQ, (ot + 1) * NQ)
                ps_f = [
                    pp.tile([128, 512], F32, name=f"psf{ot}_{c}", tag="s1",
                            bufs=2)
                    for c in range(2)
                ]
                for ft in range(4):
                    for c in range(2):
                        mm(
                            ps_f[c][:],
                            wo_sb[:, ft * D + ot * 128 : ft * D + (ot + 1) * 128],
                            ot0[:, ft * NQ + c * 512 : ft * NQ + c * 512 + 512],
                            start=(ft == 0),
                            stop=(ft == 3),
                        )
                rl = sm.tile([128, NQ], BF16, name=f"rl{ot}", tag="rl", bufs=2)
                for c in range(2):
                    nc.scalar.activation(
                        rl[:, c * 512 : (c + 1) * 512], ps_f[c][:], AF.Relu,
                        bias=bo_sb[:, ot : ot + 1],
                    )
                nc.vector.tensor_add(o1[:, osl], ot0[:, osl], rl[:])
                # square of o1 block for LN1 sumsq (sums deferred)
                sq1t = sm.tile([128, NQ], BF16, name=f"sq1t{ot}", tag="sqt",
                               bufs=4)
                sq1_tiles.append(sq1t)
                nc.vector.tensor_mul(sq1t[:], o1[:, osl], o1[:, osl])

            # ---- phase 5: LN1 -> out ------------------------------------------
            otout = ap.tile([128, 4 * NQ], F32, name="otout")
            sts1 = ln_sums(o1, sq1_tiles, "l1")
            rr1, rm1 = ln_stats_and_reps(sts1, "l1")
            for t in range(4):
                sl = slice(t * NQ, (t + 1) * NQ)
                tmp = sm.tile([128, NQ], BF16, name=f"tmp{t}", tag="rl", bufs=2)
                nc.vector.tensor_mul(tmp[:], o1[:, sl], rr1[:])
                if affine:
                    nc.vector.tensor_sub(tmp[:], tmp[:], rm1[:])
                    nc.vector.tensor_scalar(
                        otout[:, sl], tmp[:],
                        gb_sb[:, 8 + t : 8 + t + 1], gb_sb[:, 12 + t : 12 + t + 1],
                        mult, add,
                    )
                else:
                    nc.vector.tensor_sub(otout[:, sl], tmp[:], rm1[:])
                nc.sync.dma_start(
                    out_d[t * 128 : (t + 1) * 128, :], otout[:, sl]
                )

    _split_multi_waits(nc)
    return nc


_nc_cache = {}


def _get_nc(kt_tiles=5, affine=False):
    key = (kt_tiles, affine)
    if key not in _nc_cache:
        _nc_cache[key] = build_nc(kt_tiles, affine)
    return _nc_cache[key]


def _kt_tiles_for(mask):
    n = int(max(int((mask[b] != 0).sum()) for b in range(mask.shape[0])))
    return max(1, (n + 127) // 128)


def _is_affine(g0, b0, g1, b1):
    return not (
        np.all(np.asarray(g0) == 1.0)
        and np.all(np.asarray(b0) == 0.0)
        and np.all(np.asarray(g1) == 1.0)
        and np.all(np.asarray(b1) == 0.0)
    )


def prep_inputs(Q, K, mask, Wq, bq, Wk, bk, Wv, bv, Wo, bo, g0, b0, g1, b1,
                kt_tiles=None):
    f32, bf = np.float32, ml_dtypes.bfloat16
    if kt_tiles is None:
        kt_tiles = _kt_tiles_for(np.asarray(mask))
    nkp = kt_tiles * 128

    def percol(v):  # [512] feature vector -> [128, 4] per-partition layout
        return np.ascontiguousarray(np.asarray(v, f32).reshape(4, 128).T)

    wv_h = np.ascontiguousarray(
        np.vstack([np.asarray(Wv, f32), np.asarray(bv, f32)[None, :]])
    ).astype(bf)
    cr = np.zeros((2, 128), f32)
    cr[0, :] = 1.0
    cr = cr.astype(bf)
    gb = np.concatenate(
        [percol(g0), percol(b0), percol(g1), percol(b1)], axis=1
    ).astype(f32)
    cn = np.full((128, 1), 1.0 / D, f32).astype(bf)
    wq_h = np.ascontiguousarray(np.asarray(Wq, f32)).astype(bf)
    wk_h = np.ascontiguousarray(np.asarray(Wk, f32)).astype(bf)
    wo_h = np.ascontiguousarray(np.asarray(Wo, f32)).astype(bf)

    in_maps = []
    for b in range(B):
        qt = np.ascontiguousarray(np.asarray(Q[b], f32).T).astype(bf)
        idx = np.nonzero(np.asarray(mask)[b] != 0)[0]
        kc = np.zeros((nkp, D), f32)
        kc[: len(idx)] = np.asarray(K[b], f32)[idx]
        indrow = np.zeros((1, nkp), f32)
        indrow[0, : len(idx)] = 1.0
        kt = np.ascontiguousarray(np.vstack([kc.T, indrow])).astype(bf)
        ind = np.ascontiguousarray(indrow.reshape(kt_tiles, 128).T).astype(bf)
        in_maps.append(
            {
                "qt": qt,
                "kt": kt,
                "wq": wq_h,
                "wk": wk_h,
                "wv": wv_h,
                "wo": wo_h,
                "bq": percol(bq),
                "bk": percol(bk),
                "bo": percol(bo),
                "ind": ind,
                "cr": cr,
                "gb": gb,
                "cn": cn,
            }
        )
    return in_maps


def kernel(Q, K, mask, Wq, bq, Wk, bk, Wv, bv, Wo, bo, g0, b0, g1, b1):
    mask = np.asarray(mask)
    kt_tiles = _kt_tiles_for(mask)
    affine = _is_affine(g0, b0, g1, b1)
    nc = _get_nc(kt_tiles, affine)
    in_maps = prep_inputs(
        Q, K, mask, Wq, bq, Wk, bk, Wv, bv, Wo, bo, g0, b0, g1, b1, kt_tiles
    )
    res = run_bass_kernel_spmd(nc, in_maps, list(range(N_CORES)))
    out = np.stack(
        [np.ascontiguousarray(res.results[i]["out"].T) for i in range(N_CORES)]
    )
    return out.astype(np.float32)


# revision 15
# speedup vs baseline: 1.4425x; 1.1154x over previous
"""Trainium2 Bass kernel for the masked-attention block (nn_MAB_61607010894006).

Sharding: data-parallel over batch B=8 across 8 NeuronCores (one batch row
per core, weights replicated, no collectives).

v2 design (vs the 249us v1 baseline):
  - bf16 activations+weights (fp32 PSUM accumulation, fp32 LN stats, fp32
    output). Halves DMA, enables FWL on LDWEIGHTS and DVE 2x perf modes.
  - Scores: one matmul per (head, k-tile) with free dim 1024; head PAIRS
    run concurrently on the PE via row tiling (contraction=64: heads
    2t/2t+1 at array rows 0-63/64-127).
  - Masking without exp-bias: masked k tokens compacted out on the host;
    padded k columns are zero (scores=0, exp=1) and both the v rows and
    the denominator indicator row are 0 there, so pads contribute nothing
    to numerator or denominator.
  - Softmax denominator rides the po matmul as a 65th v row (indicator).
    ALL reciprocals/rsqrts are computed as exp(-ln(x)) / exp(-0.5 ln(x))
    on ScalarE: Ln and Exp share one ACT table set, so the kernel does a
    single ACT_TABLE_LOAD (v1 lost 26us to exp<->recip table thrash).
    (Custom DVE ops and gpsimd compute ops fail codegen in this
    toolchain - only gpsimd DMA works.)
  - Row->partitions broadcasts via tiny PE ones-matmuls; elementwise on
    DVE in bf16 (2x mode); odd-head partition shifts via gpsimd DMA.
"""

import sys

sys.path.insert(0, "/opt/trn_rl_repo")

import numpy as np
import ml_dtypes

import concourse.bass as bass
import concourse.mybir as mybir
import concourse.tile as tile
import concourse.bass_utils as _bass_utils
from concourse.bass_utils import run_bass_kernel_spmd

# Re-enable walrus LDWEIGHTS dedup/overlap: with weight-stationary loop
# nests (2+ matmuls per lhsT) the per-matmul serialized LDWEIGHTS is ~25%
# of PE time. Wrap run_command to flip the hardcoded --enable-ldw-opt.
if not getattr(_bass_utils, "_ldw_opt_patched", False):
    _orig_run_command = _bass_utils.run_command

    def _run_command_ldw(cmd, *a, **kw):
        if isinstance(cmd, list):
            cmd = [
                c
                for c in cmd
            ]
        return _orig_run_command(cmd, *a, **kw)

    _bass_utils.run_command = _run_command_ldw
    _bass_utils._ldw_opt_patched = True

F32 = mybir.dt.float32
F32R = mybir.dt.float32r
BF16 = mybir.dt.bfloat16
AF = mybir.ActivationFunctionType

B, NQ, NK, D, H, DH = 8, 1024, 1024, 512, 8, 64
EPS = 1e-5
N_CORES = 8


def _split_multi_waits(nc):
    """This toolchain's walrus allows ONE sem wait per TPB instruction; Tile
    can emit several (kernel-tail drain). Hoist extras onto preceding
    single-wait NOPs on the same engine stream (equivalent: in-order issue).
    """
    multi_update = []
    for fn in nc.m.functions:
        for bb in fn.blocks:
            insts = bb.instructions
            new = []
            changed = False
            for inst in insts:
                si = inst.sync_info
                if si is not None and si.on_wait and len(si.on_wait) > 1:
                    waits = list(si.on_wait)
                    for w in waits[:-1]:
                        nop = mybir.InstNoOp(
                            name=f"I-wsplit-{nc.next_id()}", engine=inst.engine
                        )
                        nop.sync_info = mybir.SyncInfo(on_wait=[w], on_update=[])
                        new.append(nop)
                    inst.sync_info = mybir.SyncInfo(
                        on_wait=[waits[-1]], on_update=list(si.on_update)
                    )
                    changed = True
                if si is not None and si.on_update and len(si.on_update) > 1:
                    multi_update.append(inst.name)
                new.append(inst)
            if changed:
                bb.instructions = new
    if multi_update:
        raise RuntimeError(f">1 sem update unsupported: {multi_update[:10]}")


def build_nc(kt_tiles=5, affine=False):
    NKP = kt_tiles * 128  # compacted+padded key/value token count
    nc = bass.Bass()

    qt_d = nc.dram_tensor("qt", [D, NQ], BF16, kind="ExternalInput")
    kt_d = nc.dram_tensor("kt", [D + 1, NKP], BF16, kind="ExternalInput")  # +ind
    wq_d = nc.dram_tensor("wq", [D, D], BF16, kind="ExternalInput")
    wk_d = nc.dram_tensor("wk", [D, D], BF16, kind="ExternalInput")
    wv_d = nc.dram_tensor("wv", [D + 1, D], BF16, kind="ExternalInput")  # +bv row
    wo_d = nc.dram_tensor("wo", [D, D], BF16, kind="ExternalInput")
    bq_d = nc.dram_tensor("bq", [128, 4], F32, kind="ExternalInput")
    bk_d = nc.dram_tensor("bk", [128, 4], F32, kind="ExternalInput")
    bo_d = nc.dram_tensor("bo", [128, 4], F32, kind="ExternalInput")
    ind_d = nc.dram_tensor("ind", [128, kt_tiles], BF16, kind="ExternalInput")
    cr_d = nc.dram_tensor("cr", [2, 128], BF16, kind="ExternalInput")  # row0=ones
    gb_d = nc.dram_tensor("gb", [128, 16], F32, kind="ExternalInput")  # percol
    cn_d = nc.dram_tensor("cn", [128, 1], BF16, kind="ExternalInput")  # 1/512
    out_d = nc.dram_tensor("out", [D, NQ], F32, kind="ExternalOutput")

    mult, add = mybir.AluOpType.mult, mybir.AluOpType.add

    def mm(out, lhsT, rhs, **kw):
        nc.tensor.matmul(out, lhsT, rhs, **kw)

    with tile.TileContext(nc) as tc:
        with (
            tc.tile_pool(name="wp", bufs=1) as wp,
            tc.tile_pool(name="ap", bufs=1) as ap,
            tc.tile_pool(name="sm", bufs=2) as sm,
            tc.tile_pool(name="pp", bufs=1, space="PSUM") as pp,
        ):
            # ---- small constants first (cheap DMAs) ---------------------------
            bq_sb = wp.tile([128, 4], F32, name="bq_sb")
            bk_sb = wp.tile([128, 4], F32, name="bk_sb")
            bo_sb = wp.tile([128, 4], F32, name="bo_sb")
            ind_sb = wp.tile([128, kt_tiles], BF16, name="ind_sb")
            cr_sb = wp.tile([2, 128], BF16, name="cr_sb")
            gb_sb = wp.tile([128, 16], F32, name="gb_sb")
            cn_sb = wp.tile([128, 1], BF16, name="cn_sb")
            nc.sync.dma_start(bq_sb[:], bq_d[:])
            nc.sync.dma_start(bk_sb[:], bk_d[:])
            nc.sync.dma_start(bo_sb[:], bo_d[:])
            nc.sync.dma_start(ind_sb[:], ind_d[:])
            nc.sync.dma_start(cr_sb[:], cr_d[:])
            nc.sync.dma_start(gb_sb[:], gb_d[:])
            nc.sync.dma_start(cn_sb[:], cn_d[:])
            eps_sb = wp.tile([1, 1], F32, name="eps_sb")
            nc.vector.memset(eps_sb[:], EPS)
            sum_lhs = cn_sb[:, 0:1]                    # [128,1] bf16 = 1/512
            ones_row = cr_sb[0:1, :]                   # [1,128] bf16 lhsT

            # ---- PE warmup while DMAs stream ---------------------------------
            wu_sb = wp.tile([128, 128], BF16, name="wu_sb")
            nc.vector.memset(wu_sb[:], 0.001)
            wu_ps = pp.tile([128, 512], F32, name="wu_ps", tag="s1", bufs=2)
            for i in range(36):
                mm(wu_ps[:, 0:128], wu_sb[:], wu_sb[:],
                   start=(i == 0), stop=(i == 35))
            wu_out = wp.tile([1, 1], F32, name="wu_out")
            nc.vector.tensor_copy(wu_out[:], wu_ps[0:1, 0:1])

            # ---- weights ------------------------------------------------------
            wq_sb = wp.tile([128, 4 * D], BF16, name="wq_sb")
            wk_sb = wp.tile([128, 4 * D], BF16, name="wk_sb")
            wv_sb = wp.tile([128, 4 * D], BF16, name="wv_sb")
            wv1_sb = wp.tile([1, D], BF16, name="wv1_sb")
            wo_sb = wp.tile([128, 4 * D], BF16, name="wo_sb")
            for t in range(4):
                nc.sync.dma_start(
                    wv_sb[:, t * D : (t + 1) * D], wv_d[t * 128 : (t + 1) * 128, :]
                )
            nc.sync.dma_start(wv1_sb[:, :], wv_d[D : D + 1, :])

            # ---- staged inputs (kt first: v+k projections start earliest) ----
            kt_sb = wp.tile([128, 4 * NKP], BF16, name="kt_sb")
            kt1_sb = wp.tile([1, NKP], BF16, name="kt1_sb")
            for t in range(4):
                nc.sync.dma_start(
                    kt_sb[:, t * NKP : (t + 1) * NKP],
                    kt_d[t * 128 : (t + 1) * 128, :],
                )
            nc.sync.dma_start(kt1_sb[:, :], kt_d[D : D + 1, :])
            for t in range(4):
                nc.sync.dma_start(
                    wk_sb[:, t * D : (t + 1) * D], wk_d[t * 128 : (t + 1) * 128, :]
                )
            qt_sb = wp.tile([128, 4 * NQ], BF16, name="qt_sb")
            for t in range(4):
                nc.sync.dma_start(
                    qt_sb[:, t * NQ : (t + 1) * NQ],
                    qt_d[t * 128 : (t + 1) * 128, :],
                )
            for t in range(4):
                nc.sync.dma_start(
                    wq_sb[:, t * D : (t + 1) * D], wq_d[t * 128 : (t + 1) * 128, :]
                )
            for t in range(4):
                nc.sync.dma_start(
                    wo_sb[:, t * D : (t + 1) * D], wo_d[t * 128 : (t + 1) * 128, :]
                )

            # ---- persistent activations --------------------------------------
            q_bf = ap.tile([128, 4 * NQ], BF16, name="q_bf")
            k_bf = ap.tile([128, 4 * NKP], BF16, name="k_bf")
            # v: per k-tile vt: 8 head blocks of 65 (64 values + indicator col)
            v_sb = ap.tile([128, kt_tiles * (8 * 65)], BF16, name="v_sb")
            v_ones = v_sb.rearrange("p (v h x) -> p v h x", v=kt_tiles, h=8)[
                :, :, :, 64
            ]
            nc.vector.tensor_copy(
                v_ones,
                ind_sb.rearrange("p (v a) -> p v a", a=1)
                .broadcast_to([128, kt_tiles, 8]),
            )

            # ---- phase 1a: v projection (token-major, +bias via ind row) -----
            for vt in range(kt_tiles):
                ps_v = pp.tile([128, 512], F32, name="ps_v", tag="s1", bufs=2)
                for kc in range(4):
                    mm(
                        ps_v[:],
                        kt_sb[:, kc * NKP + vt * 128 : kc * NKP + (vt + 1) * 128],
                        wv_sb[:, kc * D : (kc + 1) * D],
                        start=(kc == 0),
                        stop=False,
                    )
                mm(
                    ps_v[:],
                    kt1_sb[0:1, vt * 128 : (vt + 1) * 128],
                    wv1_sb[0:1, :],
                    start=False,
                    stop=True,
                )
                v_dst = v_sb[:, vt * 520 : (vt + 1) * 520].rearrange(
                    "p (h x) -> p h x", h=8
                )[:, :, 0:64]
                nc.scalar.copy(v_dst, ps_v.rearrange("p (h x) -> p h x", h=8))

            # ---- phases 1b+2 interleaved per t-block -------------------------
            o_bf = ap.tile([128, 4 * NQ], BF16, name="o_bf")
            sqt_tiles = []
            kchunks = [(0, 512), (512, NKP - 512)] if NKP > 512 else [(0, NKP)]

            for t in range(4):
                tsl = slice(t * NQ, (t + 1) * NQ)
                # -- k projection block t (kc-outer, weights reused) --
                ps_k = [
                    pp.tile([128, cw], F32, name=f"ps_k{t}_{ci}", tag="s1", bufs=2)
                    for ci, (cs, cw) in enumerate(kchunks)
                ]
                for kc in range(4):
                    for ci, (cs, cw) in enumerate(kchunks):
                        mm(
                            ps_k[ci][:],
                            wk_sb[:, kc * D + t * 128 : kc * D + (t + 1) * 128],
                            kt_sb[:, kc * NKP + cs : kc * NKP + cs + cw],
                            start=(kc == 0),
                            stop=(kc == 3),
                        )
                for ci, (cs, cw) in enumerate(kchunks):
                    nc.vector.tensor_scalar_add(
                        k_bf[:, t * NKP + cs : t * NKP + cs + cw],
                        ps_k[ci][:],
                        bk_sb[:, t : t + 1],
                    )
                # -- q projection block t --
                ps_q = [
                    pp.tile([128, 512], F32, name=f"ps_q{t}_{c}", tag="s1", bufs=2)
                    for c in range(2)
                ]
                for kc in range(4):
                    for c in range(2):
                        mm(
                            ps_q[c][:],
                            wq_sb[:, kc * D + t * 128 : kc * D + (t + 1) * 128],
                            qt_sb[:, kc * NQ + c * 512 : kc * NQ + c * 512 + 512],
                            start=(kc == 0),
                            stop=(kc == 3),
                        )
                for c in range(2):
                    nc.vector.tensor_scalar_add(
                        q_bf[:, t * NQ + c * 512 : t * NQ + c * 512 + 512],
                        ps_q[c][:],
                        bq_sb[:, t : t + 1],
                    )

                # -- scores + exp for head pair (2t, 2t+1), row-tiled --
                at_tiles = {0: [], 1: []}
                for i in range(kt_tiles):
                    for sub in range(2):
                        rh = sub * 64
                        at_sb = ap.tile(
                            [128, NQ], BF16, name=f"at{t}_{i}_{sub}", tag="at",
                            bufs=12,
                        )
                        at_tiles[sub].append(at_sb)
                        for c in range(2):
                            ps_s = pp.tile(
                                [128, 512], F32, name=f"s{t}_{i}_{sub}_{c}",
                                tag="sx", bufs=3,
                            )
                            mm(
                                ps_s[:],
                                k_bf[rh : rh + 64,
                                     t * NKP + i * 128 : t * NKP + (i + 1) * 128],
                                q_bf[rh : rh + 64,
                                     t * NQ + c * 512 : t * NQ + c * 512 + 512],
                                start=True,
                                stop=True,
                            )
                            nc.scalar.activation(
                                at_sb[:, c * 512 : (c + 1) * 512], ps_s[:],
                                AF.Exp, scale=0.125,
                            )

                # -- A@V + normalize + residual, per head --
                for sub in range(2):
                    h = 2 * t + sub
                    rh = sub * 64
                    pos = []
                    for c in range(2):
                        po = pp.tile([65, 512], F32, name=f"po{h}_{c}",
                                     tag="po1", bufs=3)
                        pos.append(po)
                        for i in range(kt_tiles):
                            mm(
                                po[:],
                                v_sb[:, i * 520 + h * 65 : i * 520 + (h + 1) * 65],
                                at_tiles[sub][i][:, c * 512 : (c + 1) * 512],
                                start=(i == 0),
                                stop=(i == kt_tiles - 1),
                            )
                    for c in range(2):
                        po = pos[c]
                        lnr = sm.tile([1, 512], F32, name=f"ln{h}{c}",
                                      tag="ri", bufs=2)
                        nc.scalar.activation(lnr[:], po[64:65, :], AF.Ln)
                        rinv = sm.tile([1, 512], BF16, name=f"r{h}{c}",
                                       tag="ri2", bufs=2)
                        nc.scalar.activation(rinv[:], lnr[:], AF.Exp,
                                             scale=-1.0)
                        pb = pp.tile([64, 512], F32, name=f"pb{h}{c}",
                                     tag="po1", bufs=3)
                        mm(pb[:], cr_sb[0:1, 0:64], rinv[:],
                           start=True, stop=True)
                        rb = sm.tile([64, 512], BF16, name=f"rb{h}{c}",
                                     tag="rb", bufs=2)
                        nc.vector.tensor_copy(rb[:], pb[:])
                        avn = sm.tile([64, 512], BF16, name=f"av{h}{c}",
                                      tag="av", bufs=2)
                        nc.vector.tensor_mul(avn[:], po[0:64, :], rb[:])
                        csl = slice(t * NQ + c * 512, t * NQ + c * 512 + 512)
                        if rh == 0:
                            nc.vector.tensor_add(
                                o_bf[0:64, csl], avn[:], q_bf[0:64, csl]
                            )
                        else:
                            av2 = sm.tile([128, 512], BF16, name=f"av2_{h}{c}",
                                          tag="av2", bufs=2)
                            nc.gpsimd.dma_start(av2[64:128, :], avn[:])
                            nc.vector.tensor_add(
                                o_bf[64:128, csl], av2[64:128, :],
                                q_bf[64:128, csl],
                            )

                # square of o block t for LN0 sumsq (sums deferred)
                sqt = sm.tile([128, NQ], BF16, name=f"sqt{t}", tag="sqt", bufs=4)
                sqt_tiles.append(sqt)
                nc.vector.tensor_mul(sqt[:], o_bf[:, tsl], o_bf[:, tsl])

            # ---- layernorm sums + stats + broadcast helper --------------------
            def ln_sums(x_bf, sq_tiles, tag):
                """Post-phase LN sums: per c-chunk one [65,512] f32 PSUM tile
                (mean row 0, mean-square row 64 via column tile position)."""
                sts = []
                for c in range(2):
                    st = pp.tile([65, 512], F32, name=f"st{tag}{c}", tag="sx",
                                 bufs=3)
                    for t in range(4):
                        csl = slice(t * NQ + c * 512, t * NQ + c * 512 + 512)
                        mm(st[0:1, :], sum_lhs, x_bf[:, csl],
                           start=(t == 0), stop=(t == 3))
                        mm(st[64:65, :], sum_lhs,
                           sq_tiles[t][:, c * 512 : c * 512 + 512],
                           start=(t == 0), stop=(t == 3))
                    sts.append(st)
                return sts

            def ln_stats_and_reps(sts, tag):
                """sts: per-c [65,512] f32 PSUM (mean row 0, meansq row 64).
                Returns (rr, rm): [128, NQ] bf16 broadcasts of rstd, mu*rstd."""
                var = sm.tile([1, NQ], F32, name=f"var{tag}", tag="var", bufs=2)
                for c in range(2):
                    csl = slice(c * 512, (c + 1) * 512)
                    nc.scalar.activation(var[:, csl], sts[c][0:1, :], AF.Square)
                    nc.vector.tensor_sub(var[:, csl], sts[c][64:65, :],
                                         var[:, csl])
                # rstd = exp(-0.5*ln(var+eps)) — stays in the Exp table set
                nc.scalar.activation(var[:], var[:], AF.Ln,
                                     bias=eps_sb[0:1, 0:1])
                rstd = sm.tile([1, NQ], BF16, name=f"rs{tag}", tag="rs", bufs=2)
                nc.scalar.activation(rstd[:], var[:], AF.Exp, scale=-0.5)
                murm = sm.tile([1, NQ], BF16, name=f"mm{tag}", tag="mm2", bufs=2)
                for c in range(2):
                    csl = slice(c * 512, (c + 1) * 512)
                    nc.vector.tensor_mul(murm[:, csl], sts[c][0:1, :],
                                         rstd[:, csl])
                rr = sm.tile([128, NQ], BF16, name=f"rrb{tag}", tag="rrb", bufs=2)
                rm = sm.tile([128, NQ], BF16, name=f"rmb{tag}", tag="rmb", bufs=2)
                for c in range(2):
                    csl = slice(c * 512, (c + 1) * 512)
                    rr_ps = pp.tile([128, 512], F32, name=f"rr{tag}{c}",
                                    tag="sx", bufs=3)
                    mm(rr_ps[:], ones_row, rstd[:, csl],
                       start=True, stop=True)
                    nc.vector.tensor_copy(rr[:, csl], rr_ps[:])
                    rm_ps = pp.tile([128, 512], F32, name=f"rm{tag}{c}",
                                    tag="sx", bufs=3)
                    mm(rm_ps[:], ones_row, murm[:, csl],
                       start=True, stop=True)
                    nc.vector.tensor_copy(rm[:, csl], rm_ps[:])
                return rr, rm

            # ---- phase 3: LN0 -------------------------------------------------
            ot0 = ap.tile([128, 4 * NQ], BF16, name="ot0")
            sts0 = ln_sums(o_bf, sqt_tiles, "l0")
            rr0, rm0 = ln_stats_and_reps(sts0, "l0")
            for t in range(4):
                sl = slice(t * NQ, (t + 1) * NQ)
                nc.vector.tensor_mul(ot0[:, sl], o_bf[:, sl], rr0[:])
                nc.vector.tensor_sub(ot0[:, sl], ot0[:, sl], rm0[:])
                if affine:
                    nc.vector.tensor_scalar(
                        ot0[:, sl], ot0[:, sl],
                        gb_sb[:, 0 + t : 0 + t + 1], gb_sb[:, 4 + t : 4 + t + 1],
                        mult, add,
                    )

            # ---- phase 4: FC + relu + residual; LN1 sums interleaved ---------
            o1 = ap.tile([128, 4 * NQ], BF16, name="o1")
            sq1_tiles = []
            for ot in range(4):
                osl = slice(ot * NQ, (ot + 1) * NQ)
                ps_f = [
                    pp.tile([128, 512], F32, name=f"psf{ot}_{c}", tag="s1",
                            bufs=2)
                    for c in range(2)
                ]
                for ft in range(4):
                    for c in range(2):
                        mm(
                            ps_f[c][:],
                            wo_sb[:, ft * D + ot * 128 : ft * D + (ot + 1) * 128],
                            ot0[:, ft * NQ + c * 512 : ft * NQ + c * 512 + 512],
                            start=(ft == 0),
                            stop=(ft == 3),
                        )
                rl = sm.tile([128, NQ], BF16, name=f"rl{ot}", tag="rl", bufs=2)
                for c in range(2):
                    nc.scalar.activation(
                        rl[:, c * 512 : (c + 1) * 512], ps_f[c][:], AF.Relu,
                        bias=bo_sb[:, ot : ot + 1],
                    )
                nc.vector.tensor_add(o1[:, osl], ot0[:, osl], rl[:])
                # square of o1 block for LN1 sumsq (sums deferred)
                sq1t = sm.tile([128, NQ], BF16, name=f"sq1t{ot}", tag="sqt",
                               bufs=4)
                sq1_tiles.append(sq1t)
                nc.vector.tensor_mul(sq1t[:], o1[:, osl], o1[:, osl])

            # ---- phase 5: LN1 -> out ------------------------------------------
            otout = ap.tile([128, 4 * NQ], F32, name="otout")
            sts1 = ln_sums(o1, sq1_tiles, "l1")
            rr1, rm1 = ln_stats_and_reps(sts1, "l1")
            for t in range(4):
                sl = slice(t * NQ, (t + 1) * NQ)
                tmp = sm.tile([128, NQ], BF16, name=f"tmp{t}", tag="rl", bufs=2)
                nc.vector.tensor_mul(tmp[:], o1[:, sl], rr1[:])
                if affine:
                    nc.vector.tensor_sub(tmp[:], tmp[:], rm1[:])
                    nc.vector.tensor_scalar(
                        otout[:, sl], tmp[:],
                        gb_sb[:, 8 + t : 8 + t + 1], gb_sb[:, 12 + t : 12 + t + 1],
                        mult, add,
                    )
                else:
                    nc.vector.tensor_sub(otout[:, sl], tmp[:], rm1[:])
                nc.sync.dma_start(
                    out_d[t * 128 : (t + 1) * 128, :], otout[:, sl]
                )

    _split_multi_waits(nc)
    return nc


_nc_cache = {}


def _get_nc(kt_tiles=5, affine=False):
    key = (kt_tiles, affine)
    if key not in _nc_cache:
        _nc_cache[key] = build_nc(kt_tiles, affine)
    return _nc_cache[key]


def _kt_tiles_for(mask):
    n = int(max(int((mask[b] != 0).sum()) for b in range(mask.shape[0])))
    return max(1, (n + 127) // 128)


def _is_affine(g0, b0, g1, b1):
    return not (
        np.all(np.asarray(g0) == 1.0)
        and np.all(np.asarray(b0) == 0.0)
        and np.all(np.asarray(g1) == 1.0)
        and np.all(np.asarray(b1) == 0.0)
    )


def prep_inputs(Q, K, mask, Wq, bq, Wk, bk, Wv, bv, Wo, bo, g0, b0, g1, b1,
                kt_tiles=None):
    f32, bf = np.float32, ml_dtypes.bfloat16
    if kt_tiles is None:
        kt_tiles = _kt_tiles_for(np.asarray(mask))
    nkp = kt_tiles * 128

    def percol(v):  # [512] feature vector -> [128, 4] per-partition layout
        return np.ascontiguousarray(np.asarray(v, f32).reshape(4, 128).T)

    wv_h = np.ascontiguousarray(
        np.vstack([np.asarray(Wv, f32), np.asarray(bv, f32)[None, :]])
    ).astype(bf)
    cr = np.zeros((2, 128), f32)
    cr[0, :] = 1.0
    cr = cr.astype(bf)
    gb = np.concatenate(
        [percol(g0), percol(b0), percol(g1), percol(b1)], axis=1
    ).astype(f32)
    cn = np.full((128, 1), 1.0 / D, f32).astype(bf)
    wq_h = np.ascontiguousarray(np.asarray(Wq, f32)).astype(bf)
    wk_h = np.ascontiguousarray(np.asarray(Wk, f32)).astype(bf)
    wo_h = np.ascontiguousarray(np.asarray(Wo, f32)).astype(bf)

    in_maps = []
    for b in range(B):
        qt = np.ascontiguousarray(np.asarray(Q[b], f32).T).astype(bf)
        idx = np.nonzero(np.asarray(mask)[b] != 0)[0]
        kc = np.zeros((nkp, D), f32)
        kc[: len(idx)] = np.asarray(K[b], f32)[idx]
        indrow = np.zeros((1, nkp), f32)
        indrow[0, : len(idx)] = 1.0
        kt = np.ascontiguousarray(np.vstack([kc.T, indrow])).astype(bf)
        ind = np.ascontiguousarray(indrow.reshape(kt_tiles, 128).T).astype(bf)
        in_maps.append(
            {
                "qt": qt,
                "kt": kt,
                "wq": wq_h,
                "wk": wk_h,
                "wv": wv_h,
                "wo": wo_h,
                "bq": percol(bq),
                "bk": percol(bk),
                "bo": percol(bo),
                "ind": ind,
                "cr": cr,
                "gb": gb,
                "cn": cn,
            }
        )
    return in_maps


def kernel(Q, K, mask, Wq, bq, Wk, bk, Wv, bv, Wo, bo, g0, b0, g1, b1):
    mask = np.asarray(mask)
    kt_tiles = _kt_tiles_for(mask)
    affine = _is_affine(g0, b0, g1, b1)
    nc = _get_nc(kt_tiles, affine)
    in_maps = prep_inputs(
        Q, K, mask, Wq, bq, Wk, bk, Wv, bv, Wo, bo, g0, b0, g1, b1, kt_tiles
    )
    res = run_bass_kernel_spmd(nc, in_maps, list(range(N_CORES)))
    out = np.stack(
        [np.ascontiguousarray(res.results[i]["out"].T) for i in range(N_CORES)]
    )
    return out.astype(np.float32)
